# revision 28
# baseline (speedup 1.0000x reference)
"""Multi-head attention Trainium2 kernel (B=8, N=1024, C=768, H=12, d=64).

Sharding: data-parallel over batch -- core b computes batch element b.

Per-core dataflow (fp16 matmul operands, fp32 PSUM accumulation; fp16 keeps
the PE on its full-clock datapath):
  - host pre-transposes x -> xT [C, N] and all weights -> [in, out] layout,
    folds the 1/sqrt(d) softmax scale into q_w, extends v_w with a zero
    column per head (slot for the softmax-denominator ones trick).
  - Qt = wqT.T @ xT   [C, N]  (transposed layout, heads on partitions)
  - Kt = wkT.T @ xT   [C, N]
  - V' = xT.T @ vwT'  [N, H*65]  (natural layout; col h*65+64 memset to 1.0)
  - per head pair t, token-chunk ch: both heads' transposed scores land in
    one 2-bank PSUM tile st[128, 2, 512]; ONE Exp activation covers the
    pair. The P@V' accumulation runs TWO m-tiles BEHIND the score stream so
    the PE never waits on an in-flight Exp (software pipeline; pair 0's
    first chunk defers its whole PV wave until the V' tiles have landed).
    yt'[d'|sum, n] = V'_h.T @ P accumulated over m-tiles; row 64 = colsum
  - per (t, ch): Yt normalized via reciprocal straight off the PSUM colsum
    rows (no staging copy), one partition-broadcast per head into the two
    halves of a single [128, 512] tile, then ONE full-width tensor_mul.
  - out = Yt.T @ pwT  [N, C], staged fp16, host casts back to fp32.

Head/tail schedule: ~36 zero matmuls warm the PE HAM clock-gate during the
input DMA stream so real matmuls run at 2.4 GHz from the start; pair-0
weights/x arrive as fine-grained tiles so the first projections chase the
DMA stream; the output projection contracts channel tile 5 last, mt4/mt5
pre-run their first 5 contraction steps inside the final ch1 chunk, and the
tail evictions ride the (idle) Scalar engine with per-half output DMAs.
"""

import numpy as np

B, N, C, H, D = 8, 1024, 768, 12, 64
KO = C // 128          # 6 contraction tiles of 128 channels
MT = N // 128          # 8 token tiles
NCH = N // 512         # 2 free-dim chunks of 512
D1 = D + 1             # V' block width per head (64 V cols + 1 ones col)
CV = H * D1            # 780 extended V channels
NCORES = 8

MM_DTYPE = "f16"

_cache = {}


def _build():
    import concourse.bacc as bacc
    import concourse.mybir as mybir
    import concourse.tile as tile

    F32 = mybir.dt.float32
    MMD = {"bf16": mybir.dt.bfloat16, "f16": mybir.dt.float16,
           "f32r": mybir.dt.float32r, "f32": mybir.dt.float32}[MM_DTYPE]
    AF = mybir.ActivationFunctionType

    nc = bacc.Bacc("TRN2", target_bir_lowering=False, debug=False)

    # wq/wk arrive host-packed: row t*128+p, col ko*128+m holds
    # W[ko*128+p, t*128+m], so each pair's weight block is one contiguous
    # 1536B line per partition instead of six 256B gathers. wv arrives as
    # three pre-grouped column splits with the ko tiles flattened per row.
    d_xT = nc.dram_tensor("xT", [C, N], MMD, kind="ExternalInput")
    d_wq = nc.dram_tensor("wqT", [C, C], MMD, kind="ExternalInput")
    d_wk = nc.dram_tensor("wkT", [C, C], MMD, kind="ExternalInput")
    d_wvP = nc.dram_tensor("wvP", [128, KO * 130], MMD, kind="ExternalInput")
    d_wvR = nc.dram_tensor("wvR", [128, KO * 260], MMD, kind="ExternalInput")
    d_wvB = nc.dram_tensor("wvB", [128, KO * 390], MMD, kind="ExternalInput")
    d_wp = nc.dram_tensor("pwT", [C, C], MMD, kind="ExternalInput")
    d_out = nc.dram_tensor("out", [N, C], MMD, kind="ExternalOutput")

    r_xT = d_xT.ap().rearrange("(ko p) n -> p ko n", p=128)
    r_wq = d_wq.ap().rearrange("(t p) m -> p t m", p=128)
    r_wk = d_wk.ap().rearrange("(t p) m -> p t m", p=128)
    r_wp = d_wp.ap().rearrange("(ko p) m -> p ko m", p=128)
    r_out = d_out.ap().rearrange("(mt p) c -> mt p c", p=128)

    with tile.TileContext(nc) as tc:
        with (
            tc.tile_pool(name="xw", bufs=1) as xw,          # xT, vwT, wp (resident)
            tc.tile_pool(name="wq", bufs=3) as wqp,         # streamed weight blocks
            tc.tile_pool(name="wk", bufs=3) as wkp,
            tc.tile_pool(name="qt", bufs=3) as qtp,         # Qt/Kt streamed per pair
            tc.tile_pool(name="kt", bufs=3) as ktp,
            tc.tile_pool(name="vp", bufs=8) as vpp,         # V' all 8 token tiles
            tc.tile_pool(name="yt", bufs=6) as ytp,         # Yt all 6 channel tiles
            tc.tile_pool(name="pp", bufs=9) as ppp,         # P = exp(St), paired
            tc.tile_pool(name="cs", bufs=4) as csp,         # recip rows
            tc.tile_pool(name="bc", bufs=3) as bcp,         # broadcast tiles
            tc.tile_pool(name="ob", bufs=3) as obp,         # output staging
            tc.tile_pool(name="mm", bufs=2, space="PSUM") as mmp,
            tc.tile_pool(name="st", bufs=2, space="PSUM") as stp,
            tc.tile_pool(name="ya", bufs=2, space="PSUM") as yap,
        ):
            # ---- resident x tiles, one DMA each (dependency tracking is
            # tile-granular). ch0 arrives as fine tiles so the first
            # projections chase the DMA stream: kos 4,5 lead (single-ko
            # tiles on the gpsimd queue), kos 0,1 ride the vector queue,
            # kos 2/3 trail the weight blocks on sync/scalar. ch1 is split
            # 2-ko per queue behind the head loads.
            x45 = [xw.tile([128, 1, 512], MMD, tag=f"x45{i}", name=f"x45{i}")
                   for i in range(2)]
            x01 = xw.tile([128, 2, 512], MMD, tag="x01", name="x01")
            x23 = [xw.tile([128, 1, 512], MMD, tag=f"x23{i}", name=f"x23{i}")
                   for i in range(2)]
            xc1a = xw.tile([128, 2, 512], MMD, tag="xc1a", name="xc1a")
            xc1b = xw.tile([128, 2, 512], MMD, tag="xc1b", name="xc1b")
            xc1c = xw.tile([128, 2, 512], MMD, tag="xc1c", name="xc1c")
            # V' weights split by column group: cols 0:130 (heads 0,1) are
            # needed by pair 0's PV wave, 130:390 (heads 2-5) by pairs 1-2,
            # 390:780 (heads 6-11) only from pair 3 on.
            t_wv_p = xw.tile([128, KO * 130], MMD, tag="wvp")
            t_wv_r = xw.tile([128, KO * 260], MMD, tag="wvr")
            t_wv_b = xw.tile([128, KO * 390], MMD, tag="wvb")
            t_wp = xw.tile([128, KO, C], MMD, tag="wpf")

            def x_ap(ko, col0, w):
                if col0 < 512:
                    if ko < 2:
                        return x01[:, ko, col0:col0 + w]
                    if ko < 4:
                        return x23[ko - 2][:, 0, col0:col0 + w]
                    return x45[ko - 4][:, 0, col0:col0 + w]
                c = col0 - 512
                if ko < 2:
                    return xc1a[:, ko, c:c + w]
                if ko < 4:
                    return xc1b[:, ko - 2, c:c + w]
                return xc1c[:, ko - 4, c:c + w]

            # (no PE warmup: the chip's power governor clamps the PLL to
            # 2.0 GHz when sustained activity rises -- extra matmuls push
            # average power over the threshold and cost far more than the
            # HAM cold-start they save)

            def make_qk(t, dma_engine, dma_engine2=None):
                """DMA the weight blocks for channel tile t and return
                (t_q, t_k, units) where units are deferred emitters, each
                a slice of a PSUM accumulation group (last also evicts)."""
                if t == 0:
                    # pair 0: per-ko-pair weight tiles in x arrival order
                    # (kos 4,5 lead -- the first matmuls only wait on 64KB
                    # of weights + the x45 tiles; then 2,3 beside x23; the
                    # 0,1 block rides behind)
                    t_wqa = wqp.tile([128, 256], MMD, tag="wq", name="wqb0a")
                    t_wqc = wqp.tile([128, 256], MMD, tag="wq2", name="wqb0c")
                    t_wqb = wqp.tile([128, 256], MMD, tag="wq3", name="wqb0b")
                    dma_engine.dma_start(out=t_wqa[:], in_=r_wq[:, 0, 512:768])
                    dma_engine.dma_start(out=t_wqc[:], in_=r_wq[:, 0, 256:512])
                    dma_engine.dma_start(out=t_wqb[:], in_=r_wq[:, 0, 0:256])
                    t_wka = wkp.tile([128, 256], MMD, tag="wk", name="wkb0a")
                    t_wkc = wkp.tile([128, 256], MMD, tag="wk2", name="wkb0c")
                    t_wkb = wkp.tile([128, 256], MMD, tag="wk3", name="wkb0b")
                    (dma_engine2 or dma_engine).dma_start(
                        out=t_wka[:], in_=r_wk[:, 0, 512:768])
                    (dma_engine2 or dma_engine).dma_start(
                        out=t_wkc[:], in_=r_wk[:, 0, 256:512])
                    (dma_engine2 or dma_engine).dma_start(
                        out=t_wkb[:], in_=r_wk[:, 0, 0:256])

                    def blk(tile_a, tile_c, tile_b, ko):
                        if ko >= 4:
                            return tile_a[:, (ko - 4) * 128:(ko - 3) * 128]
                        if ko >= 2:
                            return tile_c[:, (ko - 2) * 128:(ko - 1) * 128]
                        return tile_b[:, ko * 128:(ko + 1) * 128]

                    def wq_ap(ko):
                        return blk(t_wqa, t_wqc, t_wqb, ko)

                    def wk_ap(ko):
                        return blk(t_wka, t_wkc, t_wkb, ko)
                else:
                    t_wqb = wqp.tile([128, C], MMD, tag="wq", name=f"wqb{t}")
                    dma_engine.dma_start(out=t_wqb[:], in_=r_wq[:, t, :])
                    t_wkb = wkp.tile([128, C], MMD, tag="wk", name=f"wkb{t}")
                    (dma_engine2 or dma_engine).dma_start(
                        out=t_wkb[:], in_=r_wk[:, t, :])

                    def wq_ap(ko):
                        return t_wqb[:, ko * 128:(ko + 1) * 128]

                    def wk_ap(ko):
                        return t_wkb[:, ko * 128:(ko + 1) * 128]

                t_q = qtp.tile([128, N], MMD, tag="qt", name=f"q{t}")
                t_k = ktp.tile([128, N], MMD, tag="kt", name=f"k{t}")

                def unit(w_ap, dst, ch, nm):
                    nsl = slice(ch * 512, (ch + 1) * 512)
                    state = {}

                    def part(kos, is_first, is_last):
                        def emit():
                            if is_first:
                                state["ps"] = mmp.tile([128, 512], F32,
                                                       tag="mm", name=nm)
                            ps = state["ps"]
                            for j, ko in enumerate(kos):
                                nc.tensor.matmul(
                                    ps[:], w_ap(ko),
                                    x_ap(ko, ch * 512, 512),
                                    start=(is_first and j == 0),
                                    stop=(is_last and j == len(kos) - 1),
                                )
                            if is_last:
                                if t == 0 and dst is t_k and ch == 0:
                                    # pair 0's first scores each need only a
                                    # 128-col K slice: evict in two chunks so
                                    # the score stream starts one copy early
                                    nc.vector.tensor_copy(
                                        dst[:, nsl][:, 0:128], ps[:, 0:128])
                                    nc.vector.tensor_copy(
                                        dst[:, nsl][:, 128:512], ps[:, 128:512])
                                else:
                                    nc.vector.tensor_copy(dst[:, nsl], ps[:])
                        return emit

                    if t == 0:
                        # pair 0 contracts in x arrival order, which
                        # differs per chunk (ch0: gpsimd leads with kos
                        # 4,5; ch1: sync/scalar carry kos 0-3 first)
                        if ch == 0:
                            return [part((4, 5), True, False),
                                    part((2, 3), False, False),
                                    part((0, 1), False, True)]
                        return [part((0, 1), True, False),
                                part((2, 3), False, False),
                                part((4, 5), False, True)]
                    return [part((0, 1, 2), True, False),
                            part((3, 4, 5), False, True)]

                units = []
                units += unit(wq_ap, t_q, 0, f"pq{t}a")
                units += unit(wk_ap, t_k, 0, f"pk{t}a")
                units += unit(wq_ap, t_q, 1, f"pq{t}b")
                units += unit(wk_ap, t_k, 1, f"pk{t}b")
                return t_q, t_k, units

            # ---- head DMAs, spread across engine queues so the striped
            # DMA rings carry first-needed operands first ----
            t_q, t_k, units0 = make_qk(0, nc.sync, nc.scalar)
            nc.gpsimd.dma_start(out=x45[0][:], in_=r_xT[:, 4:5, 0:512])
            nc.gpsimd.dma_start(out=x45[1][:], in_=r_xT[:, 5:6, 0:512])
            nc.sync.dma_start(out=x23[0][:], in_=r_xT[:, 2:3, 0:512])
            nc.scalar.dma_start(out=x23[1][:], in_=r_xT[:, 3:4, 0:512])
            nc.gpsimd.dma_start(out=x01[:], in_=r_xT[:, 0:2, 0:512])
            nc.sync.dma_start(out=xc1a[:], in_=r_xT[:, 0:2, 512:1024])
            nc.scalar.dma_start(out=xc1b[:], in_=r_xT[:, 2:4, 512:1024])
            nc.gpsimd.dma_start(out=xc1c[:], in_=r_xT[:, 4:6, 512:1024])
            nc.gpsimd.dma_start(out=t_wv_p[:], in_=d_wvP.ap())
            nc.gpsimd.dma_start(out=t_wv_r[:], in_=d_wvR.ap())
            nc.gpsimd.dma_start(out=t_wv_b[:], in_=d_wvB.ap())
            # wp is not needed until the very end; its DMA is issued after
            # pair 0 so it never competes with the critical head loads

            # warm the Exp activation table while the head DMAs stream so
            # the first real exp doesn't pay the ~1.3us table load
            t_warm = xw.tile([1, 2], F32, tag="warm")
            nc.vector.memset(t_warm[0:1, 0:1], 0.0)
            nc.scalar.activation(t_warm[0:1, 1:2], t_warm[0:1, 0:1], AF.Exp)

            # pair-0 ch0 projections run first, q/k parts interleaved in
            # x-chunk arrival order; ch1 projections are woven into the
            # ch0 attention stream
            for idx in (0, 3, 1, 4, 2, 5):
                units0[idx]()
            qk_tiles = {0: (t_q, t_k)}
            created = 0
            pend = []  # (need_by_tile_idx, deferred emitter)

            # ---- V' projection units (weavable): V'[n, cv] = xT.T @ vwT ----
            t_v = [vpp.tile([128, CV], MMD, tag="v", name=f"v{mt}")
                   for mt in range(MT)]

            def v_unit(mt):
                tv = t_v[mt]

                def make_part(wv_tile, col0, width, nm):
                    def part():
                        ps = mmp.tile([128, 512], F32, tag="mm",
                                      name=f"v{mt}{nm}")
                        for ko in range(KO):
                            nc.tensor.matmul(
                                ps[:, :width], x_ap(ko, mt * 128, 128),
                                wv_tile[:, ko * width:(ko + 1) * width],
                                start=(ko == 0), stop=(ko == KO - 1),
                            )
                        nc.vector.tensor_copy(tv[:, col0:col0 + width],
                                              ps[:, :width])
                        ones = tv[:, col0:col0 + width].rearrange(
                            "p (h e) -> p h e", e=D1)[:, :, D:D + 1]
                        nc.vector.memset(ones, 1.0)
                    return part

                return (make_part(t_wv_p, 0, 130, "p"),
                        make_part(t_wv_r, 130, 260, "r"),
                        make_part(t_wv_b, 390, 390, "b"))

            # pair 0's weave: pk0b first (K ch1 is read by every score
            # m-tile >= 4), then V'p[0] (needed by the first PV), then pq0b
            # (ch1 queries), then the remaining V'p units. The V'r units
            # (heads 2-5, read by pairs 1-2) lead pair 1's weave; the V'b
            # units (heads 6-11, read from pair 3) fill pairs 1-2 where
            # the filler supply would otherwise thin out.
            v_units = [v_unit(mt) for mt in range(MT)]
            pend.extend((0, u) for u in units0[9:12])     # pk0b
            pend.append((0, v_units[0][0]))
            pend.extend((0, u) for u in units0[6:9])      # pq0b
            pend.extend((0, v_units[mt][0]) for mt in range(1, MT))
            v_r_pend = [(1, v_units[mt][1]) for mt in range(MT)]
            v_b_pend = [(2, v_units[mt][2]) for mt in range(MT)]

            t_yt = [ytp.tile([128, N], MMD, tag="yt", name=f"yt{i}")
                    for i in range(KO)]

            # ---- output projection units: out[n, c] = Yt.T @ pwT; yt
            # stationary shared by both output chunks (halves the
            # LDWEIGHTS). Contraction order puts channel tile 5 (the last
            # one normalized) at the end so the first 5 matmuls never wait
            # on the final normalize. Evictions optionally ride the Scalar
            # engine (idle at the tail); out DMAs are split per half. ----
            def oproj_unit(mt, poolA, dma_a, dma_b, act_evict=False):
                t_o = obp.tile([128, C], MMD, tag="ob", name=f"ob{mt}")
                msl = slice(mt * 128, (mt + 1) * 128)

                def evict(dst, src):
                    if act_evict:
                        nc.scalar.activation(dst, src, AF.Copy)
                    else:
                        nc.vector.tensor_copy(dst, src)

                def half_a():
                    psA = poolA.tile([128, 512], F32, tag=poolA is stp and "st"
                                     or "mm", name=f"poA{mt}")
                    for t_ in range(KO):
                        nc.tensor.matmul(
                            psA[:], t_yt[t_][:, msl], t_wp[:, t_, 0:512],
                            start=(t_ == 0), stop=(t_ == KO - 1),
                        )
                    evict(t_o[:, 0:512], psA[:])
                    dma_a.dma_start(out=r_out[mt, :, 0:512], in_=t_o[:, 0:512])

                def half_b():
                    psB = mmp.tile([128, 512], F32, tag="mm", name=f"poB{mt}")
                    for t_ in range(KO):
                        nc.tensor.matmul(
                            psB[:, 0:256], t_yt[t_][:, msl],
                            t_wp[:, t_, 512:768],
                            start=(t_ == 0), stop=(t_ == KO - 1),
                        )
                    evict(t_o[:, 512:768], psB[:, 0:256])
                    dma_b.dma_start(out=r_out[mt, :, 512:768],
                                    in_=t_o[:, 512:768])

                return [half_a, half_b]

            # mt4/mt5 split units: the first 5 contraction steps (channel
            # tiles 0-4, whose ch1 tokens are normalized by pair 4) pre-run
            # inside the final ch1 chunk -- the only stretch of the kernel
            # where the weave has no other PE filler -- and the tile-5
            # step + eviction + DMA complete during the final ch0 chunk.
            def oproj_split(mt):
                t_o = obp.tile([128, C], MMD, tag="ob", name=f"ob{mt}")
                msl = slice(mt * 128, (mt + 1) * 128)
                state = {}

                def pre_a():
                    psA = mmp.tile([128, 512], F32, tag="mm", name=f"poA{mt}")
                    state["psA"] = psA
                    for t_ in range(KO - 1):
                        nc.tensor.matmul(
                            psA[:], t_yt[t_][:, msl], t_wp[:, t_, 0:512],
                            start=(t_ == 0), stop=False,
                        )

                def fin_a():
                    psA = state["psA"]
                    nc.tensor.matmul(
                        psA[:], t_yt[KO - 1][:, msl], t_wp[:, KO - 1, 0:512],
                        start=False, stop=True,
                    )
                    nc.vector.tensor_copy(t_o[:, 0:512], psA[:])
                    nc.sync.dma_start(out=r_out[mt, :, 0:512],
                                      in_=t_o[:, 0:512])

                def fin_b():
                    psB = mmp.tile([128, 512], F32, tag="mm", name=f"poB{mt}")
                    for t_ in range(KO):
                        nc.tensor.matmul(
                            psB[:, 0:256], t_yt[t_][:, msl],
                            t_wp[:, t_, 512:768],
                            start=(t_ == 0), stop=(t_ == KO - 1),
                        )
                    nc.vector.tensor_copy(t_o[:, 512:768], psB[:, 0:256])
                    nc.scalar.dma_start(out=r_out[mt, :, 512:768],
                                        in_=t_o[:, 512:768])

                return pre_a, [fin_a, fin_b]

            pre4, fin4 = oproj_split(4)
            pre5, fin5 = oproj_split(5)

            # ---- per channel-tile: attention pair with PV software-pipelined
            # one m-tile behind the score stream; projection half-groups for
            # later pairs woven in as PE filler. QK blocks are created
            # just-in-time (pair t+1 at pair t) so the filler supply lasts
            # through the late pairs. ----

            for t in range(KO):
                cap = 1 if t == 0 else min(t + 1, KO - 1)
                while created < cap:
                    created += 1
                    # pair 1's blocks ride sync/scalar (the gpsimd queue is
                    # still draining x/wv when pair 1's weave needs them);
                    # later pairs prefetch far enough ahead for gpsimd
                    if created == 1:
                        q_, k_, us = make_qk(created, nc.sync, nc.scalar)
                    else:
                        q_, k_, us = make_qk(created, nc.gpsimd)
                    qk_tiles[created] = (q_, k_)
                    pend.extend((created, u) for u in us)
                if t == 1:
                    # V'r units must pop ahead of pair 1's PV stream (it
                    # reads heads 2,3); they lead the pend queue
                    pend = v_r_pend + pend + v_b_pend
                if t == 3:
                    nc.gpsimd.dma_start(out=t_wp[:], in_=r_wp[:])

                # the last pair runs ch1 first so ch0 (whose tokens the
                # output projection consumes first) is normalized last but
                # the projection starts on ch1 tokens immediately
                ch_order = [1, 0] if t == KO - 1 else [0, 1]
                for ch in ch_order:
                    last_chunk = (t == KO - 1 and ch == 0)
                    if last_chunk:
                        # ch1 is fully normalized now: weave its output
                        # projection into this chunk's attention stream
                        pend.extend((t, u) for u in fin4 + fin5)
                        for mt_ in (6, 7):
                            pend.extend(
                                (t, u) for u in oproj_unit(
                                    mt_, mmp, nc.sync, nc.scalar))
                    nsl = slice(ch * 512, (ch + 1) * 512)
                    yt0 = yap.tile([D1, 512], F32, tag="ya", name=f"ya{t}{ch}0")
                    yt1 = yap.tile([D1, 512], F32, tag="ya", name=f"ya{t}{ch}1")
                    p_tiles = [None] * MT
                    # pair-0 ch0 runs its whole score wave before any PV
                    # (the V' tiles PV needs are still streaming in); the
                    # steady state runs PV one m-tile behind the scores
                    first = (t == 0 and ch == ch_order[0])
                    lag = MT if first else 2
                    for step in range(MT + lag):
                        if step < MT:
                            mt = step
                            msl = slice(mt * 128, (mt + 1) * 128)
                            st = stp.tile([128, 2, 512], F32, tag="st",
                                          name=f"st{t}{ch}{mt}")
                            nc.tensor.matmul(
                                st[:, 0, :], t_k[0:64, msl], t_q[0:64, nsl],
                                start=True, stop=True, tile_position=(0, 0),
                            )
                            nc.tensor.matmul(
                                st[:, 1, :], t_k[64:128, msl], t_q[64:128, nsl],
                                start=True, stop=True, tile_position=(64, 0),
                            )
                            p = ppp.tile([128, 2, 512], MMD, tag="p",
                                         name=f"p{t}{ch}{mt}")
                            nc.scalar.activation(p[:], st[:], AF.Exp)
                            p_tiles[mt] = p
                            # pk0b lands at steps 1-3 (K ch1 must be complete
                            # before the step-4 score reads it); later steps
                            # pull V'p/pq0b forward into the exp-limited
                            # score wave so the PV wave starts sooner
                            if first and step >= 1 and pend and pend[0][0] == 0:
                                pend.pop(0)[1]()
                        if step >= lag:
                            mt = step - lag
                            # during pair-0's PV wave the remaining setup
                            # units pop just-in-time BEFORE the PV that
                            # consumes them
                            if first:
                                for _ in range(4):
                                    if pend and pend[0][0] == 0:
                                        pend.pop(0)[1]()
                            p = p_tiles[mt]
                            nc.tensor.matmul(
                                yt0[:], t_v[mt][:, (2 * t) * D1:(2 * t + 1) * D1],
                                p[:, 0, :], start=(mt == 0), stop=(mt == MT - 1),
                            )
                            nc.tensor.matmul(
                                yt1[:],
                                t_v[mt][:, (2 * t + 1) * D1:(2 * t + 2) * D1],
                                p[:, 1, :], start=(mt == 0), stop=(mt == MT - 1),
                            )
                        # the final ch1 chunk has no pend fillers left: the
                        # mt4/mt5 output-projection heads (channel tiles
                        # 0-4) fill its exp-limited stream instead
                        if t == KO - 1 and ch == 1:
                            if step == 4:
                                pre4()
                            elif step == 8:
                                pre5()
                        # weave projection part-groups into the stream;
                        # drain faster under backlog so pair boundaries
                        # don't inherit a burst of forced evictions
                        if not first:
                            for _ in range(2 if len(pend) > 12 else 1):
                                if pend:
                                    pend.pop(0)[1]()
                    # normalize this chunk: colsum rows staged to SBUF (the
                    # DVE copy handles the partition-64 -> 0 hop), fast
                    # reciprocals, one partition-broadcast per head, and
                    # the per-head multiplies ride GpSimd right behind the
                    # broadcasts in the same queue (frees the Vector engine
                    # for evictions). The final chunk's evictions ride the
                    # Scalar engine so they overlap the reciprocal chain.
                    t_cs = csp.tile([1, 2, 512], F32, tag="cs", name=f"cs{t}{ch}")
                    t_rc = csp.tile([1, 2, 512], F32, tag="rc", name=f"rc{t}{ch}")
                    nc.vector.tensor_copy(t_cs[0:1, 0, :], yt0[D:D1, :])
                    nc.vector.reciprocal_approx_fast(t_rc[0:1, 0, :],
                                                     t_cs[0:1, 0, :])
                    nc.vector.tensor_copy(t_cs[0:1, 1, :], yt1[D:D1, :])
                    nc.vector.reciprocal_approx_fast(t_rc[0:1, 1, :],
                                                     t_cs[0:1, 1, :])
                    if last_chunk:
                        nc.scalar.activation(t_yt[t][0:64, nsl], yt0[0:D, :],
                                             AF.Copy)
                        nc.scalar.activation(t_yt[t][64:128, nsl], yt1[0:D, :],
                                             AF.Copy)
                    else:
                        nc.vector.tensor_copy(t_yt[t][0:64, nsl], yt0[0:D, :])
                        nc.vector.tensor_copy(t_yt[t][64:128, nsl], yt1[0:D, :])
                    t_b0 = bcp.tile([128, 512], F32, tag="bc", name=f"b0{t}{ch}")
                    t_b1 = bcp.tile([128, 512], F32, tag="bc2", name=f"b1{t}{ch}")
                    nc.gpsimd.partition_broadcast(t_b0[0:64, :],
                                                  t_rc[0:1, 0, :])
                    nc.vector.tensor_mul(t_yt[t][0:64, nsl],
                                         t_yt[t][0:64, nsl], t_b0[0:64, :])
                    nc.gpsimd.partition_broadcast(t_b1[:], t_rc[0:1, 1, :])
                    nc.vector.tensor_mul(t_yt[t][64:128, nsl],
                                         t_yt[t][64:128, nsl],
                                         t_b1[64:128, :])
                    # anything still pending that this pair needs must land
                    # before the next chunk reads it
                    due = [pu for pu in pend if pu[0] <= t]
                    if due:
                        for i, u in due:
                            u()
                        pend = [pu for pu in pend if pu[0] > t]
                # next pair's projections must be complete before it starts
                for i, u in [pu for pu in pend if pu[0] == t + 1]:
                    u()
                pend = [pu for pu in pend if pu[0] != t + 1]
                if t + 1 < KO:
                    t_q, t_k = qk_tiles[t + 1]

            # ---- tail: output projection for the ch0 token tiles; the
            # Scalar engine (done with exps) handles the evictions so they
            # overlap the PE stream and the final normalize on Vector ----
            for mt in (0, 1, 2, 3):
                dma_a = nc.sync if mt % 2 == 0 else nc.scalar
                dma_b = nc.scalar if mt % 2 == 0 else nc.sync
                for u in oproj_unit(mt, stp, dma_a, dma_b, act_evict=True):
                    u()

    nc.compile()
    return nc


def _prep_inputs(x, head_mask, q_w, k_w, v_w, proj_w):
    import ml_dtypes

    mmnp = {"bf16": ml_dtypes.bfloat16, "f16": np.float16,
            "f32r": np.float32, "f32": np.float32}[MM_DTYPE]
    scale = np.float32(D ** -0.5)

    def pack_blocks(wT):
        # row t*128+p, col ko*128+m  <-  wT[ko*128+p, t*128+m]: each pair's
        # weight block becomes one contiguous per-partition line
        return np.ascontiguousarray(
            wT.reshape(KO, 128, KO, 128).transpose(2, 1, 0, 3).reshape(C, C))

    wqT = pack_blocks((q_w * scale).T.astype(np.float32)).astype(mmnp)
    wkT = pack_blocks(k_w.T.astype(np.float32)).astype(mmnp)
    vwT0 = np.zeros((C, CV), np.float32)
    vT = v_w.T.astype(np.float32)
    for h in range(H):
        vwT0[:, h * D1:h * D1 + D] = vT[:, h * D:(h + 1) * D]
    pwT = np.ascontiguousarray(proj_w.T).astype(mmnp)

    def pack_v(vw, c0, c1):
        # [128p, ko*(c1-c0)+c]  <-  vw[ko*128+p, c0+c]
        v3 = vw.reshape(KO, 128, CV)[:, :, c0:c1]
        return np.ascontiguousarray(
            v3.transpose(1, 0, 2).reshape(128, KO * (c1 - c0)))

    in_maps = []
    for b in range(NCORES):
        xT = np.ascontiguousarray(x[b].T).astype(mmnp)
        # fold head_mask^2 into this core's V weights (ones cols stay 0->1)
        vwT = vwT0.copy()
        for h in range(H):
            vwT[:, h * D1:h * D1 + D] *= head_mask[b, h] ** 2
        in_maps.append(
            {"xT": xT, "wqT": wqT, "wkT": wkT,
             "wvP": pack_v(vwT, 0, 130).astype(mmnp),
             "wvR": pack_v(vwT, 130, 390).astype(mmnp),
             "wvB": pack_v(vwT, 390, 780).astype(mmnp),
             "pwT": pwT}
        )
    return in_maps


def _run(inputs, trace=False):
    from concourse.bass_utils import run_bass_kernel_spmd

    x = np.asarray(inputs["x"], np.float32)
    head_mask = np.asarray(inputs["head_mask"], np.float32)
    in_maps = _prep_inputs(
        x,
        head_mask,
        np.asarray(inputs["q_w"], np.float32),
        np.asarray(inputs["k_w"], np.float32),
        np.asarray(inputs["v_w"], np.float32),
        np.asarray(inputs["proj_w"], np.float32),
    )
    # biases are zero by construction of this problem (spec fill=zeros);
    # q_b/k_b/v_b/proj_b are validated and otherwise unused.
    for name in ("q_b", "k_b", "v_b", "proj_b"):
        bias = np.asarray(inputs[name])
        if np.abs(bias).max() > 0:
            raise NotImplementedError(f"nonzero {name} not supported")

    if "nc" not in _cache:
        _cache["nc"] = _build()
    nc = _cache["nc"]
    res = run_bass_kernel_spmd(
        nc, in_maps, core_ids=list(range(NCORES)), trace=trace
    )
    out = np.stack([res.results[b]["out"] for b in range(NCORES)], axis=0)
    return out.astype(np.float32), res


def kernel(**inputs):
    out, _ = _run(inputs, trace=False)
    return out
